# revision 1
# baseline (speedup 1.0000x reference)
"""Single-head attention kernel for Trainium2 (Bass/Tile), 8-core data-parallel.

Problem: h [8, 4096, 96] f32; Wq/Wk/Wv [96, 96]; bq/bk/bv [96].
  Q = h @ Wq.T + bq ; K = h @ Wk.T + bk ; V = h @ Wv.T + bv
  out = softmax(Q K^T / sqrt(96)) @ V

Sharding: batch dim across the 8 NeuronCores (1 batch element per core),
params replicated. Each core runs a flash-style attention over its
[4096, 96] slice; full output gathered on host.

Per-core layout strategy (B=1, S=4096, D=96):
  - h~^T [97, S] in SBUF: h transposed (PE transposes) + a ones row, so the
    projection matmuls fold the bias add: [W^T; b].T @ [h^T; 1] = (hW^T + b)^T.
  - Q^T, K^T [96, S]: Q^T = Wq~^T.T @ h~^T (Q scaled by 1/sqrt(D) on the
    PSUM->SBUF copy). scores^T tile [j, i] = (K^T slab).T @ (Q^T chunk).
  - softmax without max-subtraction (scores ~ N(0,1); max ~ 6 -> exp safe in
    f32). One ACT exp per (i-chunk, j-tile): PSUM [128, NI] -> SBUF.
  - V~ [s, 97] with a ones column: PV matmul V~_j.T @ expS^T accumulates
    out^T [97, NI] in PSUM over j; row 96 = softmax denominators for free.
  - Epilogue: PE-transpose out^T chunks, DVE reciprocal + tensor_scalar_mul
    to normalize, DMA [128, 96] blocks to HBM.
  - All matmuls run in float32r (TF32-like, full PE rate at N>=256;
    fp32 proper costs 4 cycles/row). End-to-end rel err ~2.5e-4.
  - Emission is software-pipelined: prologue (h transposes, projections)
    interleaves into chunk 0, PV trails exp by 2 iterations, epilogues
    defer into the next chunk, so PE and ACT both stay dense.
"""

import functools
import math

import numpy as np

import concourse.mybir as mybir
import concourse.tile as tile
from concourse import bacc
from concourse.bass import ts
from concourse.bass_utils import run_bass_kernel_spmd

S = 4096
D = 96
P = 128              # s-tile (partition) size
NI = 1024            # i-chunk size (columns of scores^T per inner block)
N_CORES = 8
F32 = mybir.dt.float32
F32R = mybir.dt.float32r
AF = mybir.ActivationFunctionType


def build_attention_kernel(tc, out_dram, h, Wq, bq, Wk, bk, Wv, bv,
                           s=S, use_f32r=True):
    nc = tc.nc
    nj = s // P           # number of 128-row j tiles
    ni = min(NI, s)       # i-chunk size
    ni_chunks = s // ni   # number of i chunks
    n512 = ni // 512      # 512-wide matmuls per chunk
    scale = 1.0 / math.sqrt(D)
    # Tensors feeding fp32r matmuls must be *written* as float32r (the BIR
    # verifier enforces producer-side rounding), so allocate them in that
    # dtype directly.
    MMDT = F32R if use_f32r else F32

    from contextlib import ExitStack
    with ExitStack() as ctx:
        singles = ctx.enter_context(tc.tile_pool(name="singles", bufs=1))
        tmp = ctx.enter_context(tc.tile_pool(name="tmp", bufs=10))
        expp = ctx.enter_context(tc.tile_pool(name="expp", bufs=5))
        epi = ctx.enter_context(tc.tile_pool(name="epi", bufs=2))
        outp = ctx.enter_context(tc.tile_pool(name="outp", bufs=3))
        # One shared PSUM pool: 3 slots x 2 banks (scores tiles and all
        # small transpose/projection tiles share slots) + the accumulator
        # (2 banks) = exactly 8 banks. 3 score slots let ACT's exp stream
        # run back-to-back instead of ping-ponging with the PE.
        psp = ctx.enter_context(
            tc.tile_pool(name="psp", bufs=3, space="PSUM"))
        ps_acc = ctx.enter_context(
            tc.tile_pool(name="ps_acc", bufs=1, space="PSUM"))
        # Identity for PE transposes: embedded in the NEFF, DMA'd at t=0
        # (generating it on GpSimd costs ~6us of dead time at kernel start).
        ident_dram = nc.inline_tensor(np.eye(P, dtype=np.float32),
                                      name="ident_const")
        ident = singles.tile([P, P], F32)
        nc.sync.dma_start(out=ident, in_=ident_dram.ap())

        # --- persistent tensors ---
        # All matmul operands are MMDT (float32r): full PE rate at N>=256.
        # The BIR verifier requires every producer writing them to round to
        # f32r, so DMA'd values (biases, ones) are staged in f32 and
        # copy-cast by DVE.
        hT = singles.tile([D + 1, s], MMDT)      # h~^T (row 96 = ones)
        QT = singles.tile([D, s], MMDT)          # (Q + bq)^T / sqrt(D)
        KT = singles.tile([D, s], MMDT)
        Vt = singles.tile([P, nj, D + 1], MMDT)  # V~ tiles (col 96 = ones)
        ones_col = singles.tile([P, 1], F32)
        nc.vector.memset(ones_col, 1.0)
        nc.vector.tensor_copy(Vt[:, :, D], ones_col.broadcast_to((P, nj)))

        # --- augmented transposed weights W~^T [97, 96] (row 96 = bias) ---
        # DMAs for all weights issued first so the PE transposes never wait.
        w_sbs = []
        for W, b_ in ((Wq, bq), (Wk, bk), (Wv, bv)):
            w_sb = tmp.tile([D, D], F32, tag=f"w_sb{len(w_sbs)}")
            nc.sync.dma_start(out=w_sb, in_=W)
            b_sb = tmp.tile([1, D], F32, tag=f"b_sb{len(w_sbs)}")
            nc.sync.dma_start(out=b_sb, in_=b_.unsqueeze(0))
            w_sbs.append((w_sb, b_sb))
        wts = []
        for w_sb, b_sb in w_sbs:
            ps_w = psp.tile([D, D], F32, tag="ps")
            nc.tensor.transpose(ps_w, w_sb, ident[0:D, 0:D])
            wt = singles.tile([D + 1, D], MMDT, tag=f"wt{len(wts)}")
            nc.vector.tensor_copy(wt[0:D, :], ps_w)
            nc.vector.tensor_copy(wt[D:D + 1, :], b_sb)
            wts.append(wt)
        wqt, wkt, wvt = wts

        # --- emission helpers (prologue work, interleavable) ---
        def emit_ones_row(n):
            # h~^T ones-row segment [1, 512] (DVE is idle early; memset
            # can't write f32r).
            nc.vector.tensor_copy(
                hT[D:D + 1, ts(n, 512)],
                ones_col[0:1, 0:1].broadcast_to((1, 512)))

        def emit_transpose(j):
            h_sb = tmp.tile([P, D], F32, tag="h_sb")
            nc.sync.dma_start(out=h_sb, in_=h[ts(j, P), :])
            ps_t = psp.tile([D, P], F32, tag="ps")
            nc.tensor.transpose(ps_t, h_sb, ident)
            nc.vector.tensor_copy(hT[0:D, ts(j, P)], ps_t)

        def emit_qt_proj(n):
            ps_q = psp.tile([D, 512], F32, tag="ps")
            nc.tensor.matmul(ps_q, lhsT=wqt, rhs=hT[:, ts(n, 512)],
                             start=True, stop=True)
            nc.vector.tensor_scalar_mul(QT[:, ts(n, 512)], ps_q, scale)

        def emit_kt_proj(n):
            ps_k = psp.tile([D, 512], F32, tag="ps")
            nc.tensor.matmul(ps_k, lhsT=wkt, rhs=hT[:, ts(n, 512)],
                             start=True, stop=True)
            nc.vector.tensor_copy(KT[:, ts(n, 512)], ps_k)

        def emit_v_proj(j):
            ps_v = psp.tile([P, D], F32, tag="ps")
            nc.tensor.matmul(ps_v, lhsT=hT[:, ts(j, P)], rhs=wvt,
                             start=True, stop=True)
            nc.vector.tensor_copy(Vt[:, j, 0:D], ps_v)

        # --- prologue: just enough for chunk 0 to start ---
        n512s = s // 512          # projection matmul count per Q/K
        pre_t = min(12, nj)       # h slabs transposed up front
        pre_kt = min(3, n512s)    # K^T chunks up front
        pre_v = min(8, nj)        # V tiles up front
        for n in range(n512s):
            emit_ones_row(n)
        for j in range(pre_t):
            emit_transpose(j)
        for n in range(pre_kt):
            emit_kt_proj(n)
        for n in range(min(2, n512s)):
            emit_qt_proj(n)
        for j in range(pre_v):
            emit_v_proj(j)
        qt_emitted = min(2, n512s)

        # --- flash attention main loop ---
        # Prologue remainder is interleaved into chunk 0; each chunk's
        # epilogue (transpose + normalize + store) is interleaved into the
        # next chunk's early iterations so the PE never drains.
        pending_epilogue = []

        def emit_epilogue_unit(ic_prev, oT, c):
            i0p = ic_prev * ni
            ps_tr = psp.tile([P, D + 1], F32, tag="ps")
            nc.tensor.transpose(ps_tr, oT[:, ts(c, P)],
                                ident[0:D + 1, 0:D + 1])
            rec = outp.tile([P, 1], F32, tag="rec")
            nc.vector.reciprocal(rec, ps_tr[:, D:D + 1])
            o_sb = outp.tile([P, D], F32, tag="o_sb")
            nc.vector.tensor_scalar_mul(o_sb, ps_tr[:, 0:D], rec)
            nc.sync.dma_start(
                out=out_dram[i0p + c * P:i0p + (c + 1) * P, :], in_=o_sb)

        for ic in range(ni_chunks):
            i0 = ic * ni
            ps_o = ps_acc.tile([D + 1, ni], F32)

            def extras(j, ic=ic):
                nonlocal qt_emitted
                if ic == 0:
                    if j + pre_t < nj:
                        emit_transpose(j + pre_t)
                    if (j + 8) % 4 == 0 and pre_kt <= (j + 8) // 4 < n512s:
                        emit_kt_proj((j + 8) // 4)
                    if j + pre_v < nj:
                        emit_v_proj(j + pre_v)
                if j in (nj // 2, 3 * nj // 4) and \
                        qt_emitted < min(2 * (ic + 2), n512s):
                    emit_qt_proj(qt_emitted)
                    qt_emitted += 1

            def scores_of(j, i0=i0):
                ps_s = psp.tile([P, ni], F32, tag="ps")
                for n in range(n512):
                    nc.tensor.matmul(
                        ps_s[:, ts(n, 512)],
                        lhsT=KT[:, ts(j, P)],
                        rhs=QT[:, i0 + n * 512:i0 + (n + 1) * 512],
                        start=True, stop=True)
                e_t = expp.tile([P, ni], MMDT, tag="exp")
                nc.scalar.activation(out=e_t, in_=ps_s, func=AF.Exp)
                return e_t

            def pv_of(j, e_t, ps_o=ps_o):
                for n in range(n512):
                    nc.tensor.matmul(
                        ps_o[:, ts(n, 512)],
                        lhsT=Vt[:, j, :],
                        rhs=e_t[:, ts(n, 512)],
                        start=(j == 0), stop=(j == nj - 1))

            # PV trails scores/exp by 2 iterations: the exp pipeline stays
            # full and PV never waits on ACT.
            LAG = 3
            exp_tiles = [None] * nj
            for j in range(nj):
                exp_tiles[j] = scores_of(j)
                extras(j)
                if pending_epilogue and 2 <= j <= 1 + ni // P:
                    emit_epilogue_unit(*pending_epilogue.pop(0))
                if j >= LAG:
                    pv_of(j - LAG, exp_tiles[j - LAG])
                    exp_tiles[j - LAG] = None
            for j in range(nj - LAG, nj):
                pv_of(j, exp_tiles[j])

            # Copy the accumulator out (releases psO for the next chunk).
            # Last chunk: emit each normalize/store unit right after its
            # column block is copied, so the tail pipelines instead of
            # draining serially. Other chunks: defer units into the next
            # chunk's early iterations.
            last = ic == ni_chunks - 1
            oT = epi.tile([D + 1, ni], F32, tag="oT")
            for c in range(ni // P):
                nc.vector.tensor_copy(oT[:, ts(c, P)], ps_o[:, ts(c, P)])
                if last:
                    emit_epilogue_unit(ic, oT, c)
                else:
                    pending_epilogue.append((ic, oT, c))

        while pending_epilogue:
            emit_epilogue_unit(*pending_epilogue.pop(0))


@functools.lru_cache(maxsize=None)
def _build_module(s=S, use_f32r=True):
    nc = bacc.Bacc("TRN2", target_bir_lowering=False, debug=False,
                   num_devices=N_CORES)
    h = nc.dram_tensor("h", [s, D], F32, kind="ExternalInput").ap()
    Wq = nc.dram_tensor("Wq", [D, D], F32, kind="ExternalInput").ap()
    bq = nc.dram_tensor("bq", [D], F32, kind="ExternalInput").ap()
    Wk = nc.dram_tensor("Wk", [D, D], F32, kind="ExternalInput").ap()
    bk = nc.dram_tensor("bk", [D], F32, kind="ExternalInput").ap()
    Wv = nc.dram_tensor("Wv", [D, D], F32, kind="ExternalInput").ap()
    bv = nc.dram_tensor("bv", [D], F32, kind="ExternalInput").ap()
    out = nc.dram_tensor("out", [s, D], F32, kind="ExternalOutput").ap()
    with tile.TileContext(nc) as tc:
        build_attention_kernel(tc, out, h, Wq, bq, Wk, bk, Wv, bv,
                               s=s, use_f32r=use_f32r)
    nc.compile()
    return nc


def _run(inputs, trace=False, use_f32r=True):
    nc = _build_module(S, use_f32r)
    arrs = {k: np.ascontiguousarray(np.asarray(v), dtype=np.float32)
            for k, v in inputs.items()}
    in_maps = []
    for b_ in range(N_CORES):
        in_maps.append({
            "h": arrs["h"][b_],
            "Wq": arrs["Wq"], "bq": arrs["bq"],
            "Wk": arrs["Wk"], "bk": arrs["bk"],
            "Wv": arrs["Wv"], "bv": arrs["bv"],
        })
    res = run_bass_kernel_spmd(nc, in_maps, core_ids=list(range(N_CORES)),
                               trace=trace)
    out = np.stack([res.results[b_]["out"] for b_ in range(N_CORES)], axis=0)
    return out, res


def kernel(**inputs):
    out, _ = _run(inputs, trace=False)
    return out


def kernel_profiled(trace=True, use_f32r=True, **inputs):
    out, res = _run(inputs, trace=trace, use_f32r=use_f32r)
    return out, res



# revision 5
# speedup vs baseline: 1.0803x; 1.0803x over previous
"""Single-head attention kernel for Trainium2 (Bass/Tile), 8-core data-parallel.

Problem: h [8, 4096, 96] f32; Wq/Wk/Wv [96, 96]; bq/bk/bv [96].
  Q = h @ Wq.T + bq ; K = h @ Wk.T + bk ; V = h @ Wv.T + bv
  out = softmax(Q K^T / sqrt(96)) @ V

Sharding: batch dim across the 8 NeuronCores (1 batch element per core),
params replicated. Each core runs a flash-style attention over its
[4096, 96] slice; full output gathered on host.

Per-core design (S=4096, D=96). The kernel is ACT(exp)-bound: 16.7M exps
at 1 elem/lane/cycle @1.2GHz = 109us floor, so everything else hides
under the exp stream and the instruction count on ACT is minimized.

  - A-trick: S*sqrt(D) = Q K^T = h~ (W~q W~k^T) h~^T with h~ = [h, 1].
    A~ = W~q W~k^T / sqrt(D) is a single tiny on-chip matmul of the
    augmented weights ([Wq | bq-col] etc.); G~^T = A~^T h~^T replaces
    separate Q and K projections (one 4096-row projection, not two).
  - All big matmuls run fp16 operands (1 cycle/row at any free size, FWL
    weight loads); PSUM stays f32 (TRN2 requires f32 matmul dst).
  - scores^T tile [j, i] = (h~^T j-slab).T @ G~^T i-cols. Groups of 3
    j-tiles share one 3-bank PSUM slot so each exp ACTIVATE covers
    N=1536 (amortizes ACT's ~222-cycle access bubble); 2 slots ping-pong.
  - exp -> e_sb ring (fp16, 2-chunk ping-pong) feeds PV as the
    *stationary* operand: out[i-tile] [128, 97] accumulates over j with
    rhs = V~ tiles [128, 97] (ones column computed by the projection),
    so PV streams 97 rows per j-tile (128/97 cheaper than the [97, i]
    layout) and the output lands in natural [i, e] order: no epilogue
    transposes. Column 96 = softmax denominator for free.
  - Epilogue per i-tile: DVE reciprocal + per-row scalar mul, DMA out.
  - Softmax without max-subtraction (scores ~ N(0,1), max ~ 5.2; exp
    fits f32/fp16 comfortably). End-to-end rel err ~4e-4.
  - Emission is software-pipelined: h-slab transposes/V~ projection ride
    chunk 0, G~^T segment c+1 rides chunk c, PV of chunk c-1 interleaves
    into chunk c's score stream so PE stays dense and HAM-warm.
"""

import functools
import math

import numpy as np

import concourse.mybir as mybir
import concourse.tile as tile
from concourse import bacc
from concourse.bass import ts
from concourse.bass_utils import run_bass_kernel_spmd

S = 4096
D = 96
P = 128              # j/i tile (partition) size
NI = 512             # i-chunk width (columns per scores matmul)
JG = 3               # j-tiles per exp group (3 PSUM banks -> N=1536)
N_CORES = 8
F32 = mybir.dt.float32
F16 = mybir.dt.float16
AF = mybir.ActivationFunctionType


def build_attention_kernel(tc, out_dram, h, Wq, bq, Wk, bk, Wv, bv, s=S):
    nc = tc.nc
    nj = s // P                    # 32 j-tiles
    nchunks = s // NI              # 8 i-chunks
    ntile = NI // P                # 4 i-tiles per chunk
    njg = (nj + JG - 1) // JG      # 11 exp groups per chunk (last ragged)
    nseg = s // NI                 # G~^T segments (one per chunk)
    scale = 1.0 / math.sqrt(D)

    def jtiles(g):
        return range(g * JG, min((g + 1) * JG, nj))

    from contextlib import ExitStack
    with ExitStack() as ctx:
        singles = ctx.enter_context(tc.tile_pool(name="singles", bufs=1))
        hp = ctx.enter_context(tc.tile_pool(name="hp", bufs=8))
        op = ctx.enter_context(tc.tile_pool(name="op", bufs=4))
        # PSUM: 2 x 3-bank score slots + 2 x 1-bank PV/misc slots = 8 banks.
        scp = ctx.enter_context(tc.tile_pool(name="scp", bufs=2, space="PSUM"))
        pvp = ctx.enter_context(tc.tile_pool(name="pvp", bufs=2, space="PSUM"))

        ident_dram = nc.inline_tensor(np.eye(P, dtype=np.float32),
                                      name="ident_const")
        ident = singles.tile([P, P], F32)
        nc.sync.dma_start(out=ident, in_=ident_dram.ap())
        ones_dram = nc.inline_tensor(np.ones((1, s), dtype=np.float16),
                                     name="ones_row")

        # --- persistent SBUF tensors ---
        hT = singles.tile([D + 1, s], F16)        # h~^T (row 96 = ones)
        GT = singles.tile([D + 1, s], F16)        # G~^T = A~^T h~^T
        Vt = singles.tile([P, nj, D + 1], F16)    # V~ tiles (col 96 = ones)
        # e_sb ring: [chunk parity][group][JG*NI cols] of exp(scores^T) fp16
        e_sb = singles.tile([P, 2, njg, JG * NI], F16)
        ones_col = singles.tile([P, 1], F32)
        nc.vector.memset(ones_col, 1.0)
        nc.sync.dma_start(out=hT[D:D + 1, :], in_=ones_dram.ap())

        # --- weights: augmented transposed forms, no host-side prep ---
        # waq/wak [96, 97] f32: cols 0-95 = Wq/Wk (natural), col 96 = bias.
        # R [97, 97] fp16: rows 0-95 = Wv^T, row 96 = bv, col 96 = e_96.
        waq = singles.tile([D, D + 1], F32)
        wak = singles.tile([D, D + 1], F32)
        wv_sb = singles.tile([D, D], F32)
        bias_sb = singles.tile([2, D], F32)
        bv_sb = singles.tile([1, D], F32)
        nc.sync.dma_start(out=waq[:, 0:D], in_=Wq)
        nc.sync.dma_start(out=wak[:, 0:D], in_=Wk)
        nc.sync.dma_start(out=wv_sb, in_=Wv)
        nc.sync.dma_start(out=bias_sb[0:1, :], in_=bq.unsqueeze(0))
        nc.sync.dma_start(out=bias_sb[1:2, :], in_=bk.unsqueeze(0))
        nc.sync.dma_start(out=bv_sb, in_=bv.unsqueeze(0))

        # bias columns via one PE transpose of [2, 96]
        ps_b = pvp.tile([D, 2], F32, tag="ps")
        nc.tensor.transpose(ps_b, bias_sb, ident[0:2, 0:2])
        nc.vector.tensor_copy(waq[:, D:D + 1], ps_b[:, 0:1])
        nc.vector.tensor_copy(wak[:, D:D + 1], ps_b[:, 1:2])

        # A~ = (W~q W~k^T) * scale -> fp16 [97, 97]
        ps_a = pvp.tile([D + 1, D + 1], F32, tag="ps")
        nc.tensor.matmul(ps_a, lhsT=waq, rhs=wak, start=True, stop=True)
        A16 = singles.tile([D + 1, D + 1], F16)
        nc.vector.tensor_scalar_mul(A16, ps_a, scale)

        # R for the V~ projection
        R = singles.tile([D + 1, D + 1], F16)
        nc.vector.memset(R, 0.0)
        ps_w = pvp.tile([D, D], F32, tag="ps")
        nc.tensor.transpose(ps_w, wv_sb, ident[0:D, 0:D])
        nc.vector.tensor_copy(R[0:D, 0:D], ps_w)
        nc.vector.tensor_copy(R[D:D + 1, 0:D], bv_sb)
        nc.vector.tensor_copy(R[D:D + 1, D:D + 1],
                              ones_col[0:1, 0:1].broadcast_to((1, 1)))

        # --- emission helpers ---
        def emit_transpose(jt):
            h_sb = hp.tile([P, D], F32, tag="h_sb")
            nc.sync.dma_start(out=h_sb, in_=h[ts(jt, P), :])
            ps_t = pvp.tile([D, P], F32, tag="ps")
            nc.tensor.transpose(ps_t, h_sb, ident)
            nc.vector.tensor_copy(hT[0:D, ts(jt, P)], ps_t)

        def emit_g_seg(seg):
            ps_g = pvp.tile([D + 1, NI], F32, tag="ps")
            nc.tensor.matmul(ps_g, lhsT=A16, rhs=hT[:, ts(seg, NI)],
                             start=True, stop=True)
            nc.vector.tensor_copy(GT[:, ts(seg, NI)], ps_g)

        def emit_v_proj(jt):
            ps_v = pvp.tile([P, D + 1], F32, tag="ps")
            nc.tensor.matmul(ps_v, lhsT=hT[:, ts(jt, P)], rhs=R,
                             start=True, stop=True)
            nc.vector.tensor_copy(Vt[:, jt, :], ps_v)

        def pv_units(c):
            """Yield callables for chunk c's PV + epilogue (4 i-tiles)."""
            par = c % 2
            for t in range(ntile):
                acc = pvp.tile([P, D + 1], F32, tag="ps")
                for j in range(nj):
                    g, jj = j // JG, j % JG
                    e_sl = e_sb[:, par, g, jj * NI + t * P:
                                jj * NI + (t + 1) * P]
                    yield functools.partial(
                        nc.tensor.matmul, acc, lhsT=e_sl, rhs=Vt[:, j, :],
                        start=(j == 0), stop=(j == nj - 1))

                def epilogue(c=c, t=t, acc=acc):
                    rec = op.tile([P, 1], F32, tag="rec")
                    nc.vector.reciprocal(rec, acc[:, D:D + 1])
                    o_sb = op.tile([P, D], F32, tag="o_sb")
                    nc.vector.tensor_scalar_mul(o_sb, acc[:, 0:D], rec)
                    i0 = c * NI + t * P
                    nc.sync.dma_start(out=out_dram[i0:i0 + P, :], in_=o_sb)
                yield epilogue

        # --- prologue: enough for chunk 0 group 0 ---
        for jt in range(2 * JG):            # transposes for groups 0-1
            emit_transpose(jt)
        emit_g_seg(0)

        # --- main loop ---
        pv_iter = None
        for c in range(nchunks):
            if c > 0:
                pv_iter = pv_units(c - 1)
            for g in range(njg):
                jts = list(jtiles(g))
                sc = scp.tile([P, JG * NI], F32, tag="sc")
                for jj, jt in enumerate(jts):
                    nc.tensor.matmul(sc[:, ts(jj, NI)],
                                     lhsT=hT[:, ts(jt, P)],
                                     rhs=GT[:, ts(c, NI)],
                                     start=True, stop=True)
                width = len(jts) * NI
                nc.scalar.activation(out=e_sb[:, c % 2, g, 0:width],
                                     in_=sc[:, 0:width], func=AF.Exp)
                # interleaved extras keep PE dense without starving ACT
                if c == 0:
                    for jt in jtiles(g + 2):
                        emit_transpose(jt)
                    for jt in jts:
                        emit_v_proj(jt)
                if g == 5 and c + 1 < nseg:
                    emit_g_seg(c + 1)
                if pv_iter is not None:
                    for u in (x for _, x in zip(range(12), pv_iter)):
                        u()
        # tail: drain PV of the last chunk
        for u in pv_units(nchunks - 1):
            u()


@functools.lru_cache(maxsize=None)
def _build_module(s=S):
    nc = bacc.Bacc("TRN2", target_bir_lowering=False, debug=False,
                   num_devices=N_CORES)
    h = nc.dram_tensor("h", [s, D], F32, kind="ExternalInput").ap()
    Wq = nc.dram_tensor("Wq", [D, D], F32, kind="ExternalInput").ap()
    bq = nc.dram_tensor("bq", [D], F32, kind="ExternalInput").ap()
    Wk = nc.dram_tensor("Wk", [D, D], F32, kind="ExternalInput").ap()
    bk = nc.dram_tensor("bk", [D], F32, kind="ExternalInput").ap()
    Wv = nc.dram_tensor("Wv", [D, D], F32, kind="ExternalInput").ap()
    bv = nc.dram_tensor("bv", [D], F32, kind="ExternalInput").ap()
    out = nc.dram_tensor("out", [s, D], F32, kind="ExternalOutput").ap()
    with tile.TileContext(nc) as tc:
        build_attention_kernel(tc, out, h, Wq, bq, Wk, bk, Wv, bv, s=s)
    nc.compile()
    return nc


def _run(inputs, trace=False):
    nc = _build_module(S)
    arrs = {k: np.ascontiguousarray(np.asarray(v), dtype=np.float32)
            for k, v in inputs.items()}
    in_maps = []
    for b_ in range(N_CORES):
        in_maps.append({
            "h": arrs["h"][b_],
            "Wq": arrs["Wq"], "bq": arrs["bq"],
            "Wk": arrs["Wk"], "bk": arrs["bk"],
            "Wv": arrs["Wv"], "bv": arrs["bv"],
        })
    res = run_bass_kernel_spmd(nc, in_maps, core_ids=list(range(N_CORES)),
                               trace=trace)
    out = np.stack([res.results[b_]["out"] for b_ in range(N_CORES)], axis=0)
    return out, res


def kernel(**inputs):
    out, _ = _run(inputs, trace=False)
    return out


def kernel_profiled(trace=True, **inputs):
    out, res = _run(inputs, trace=trace)
    return out, res


# revision 6
# speedup vs baseline: 1.1514x; 1.0657x over previous
"""Single-head attention kernel for Trainium2 (Bass/Tile), 8-core data-parallel.

Problem: h [8, 4096, 96] f32; Wq/Wk/Wv [96, 96]; bq/bk/bv [96].
  Q = h @ Wq.T + bq ; K = h @ Wk.T + bk ; V = h @ Wv.T + bv
  out = softmax(Q K^T / sqrt(96)) @ V

Sharding: batch dim across the 8 NeuronCores (1 batch element per core),
params replicated. Each core runs a flash-style attention over its
[4096, 96] slice; full output gathered on host.

Per-core design (S=4096, D=96). The kernel is ACT(exp)-bound: 16.7M exps
at 1 elem/lane/cycle @1.2GHz = 109us floor, so everything else must hide
under the exp stream and ACT's per-instruction overhead (~222-cycle
access bubble) is amortized with wide ACTIVATEs.

  - A-trick: S*sqrt(D) = Q K^T = h~ (W~q W~k^T) h~^T with h~ = [h, 1].
    A~ = W~q W~k^T / sqrt(D) is a single tiny on-chip matmul of the
    augmented weights ([Wq | bq-col] etc.); G~^T = A~^T h~^T replaces
    separate Q and K projections (one 4096-row projection, not two).
  - All big matmuls run fp16 operands (1 cycle/row at any free size; f32
    PSUM dst as TRN2 requires). Rel err ~4e-4.
  - scores^T tile [j, i] = (h~^T j-slab).T @ G~^T i-cols, in i-chunks of
    512. Groups of 3 j-tiles share one 3-bank PSUM slot so each exp
    ACTIVATE covers N=1536; 2 slots ping-pong (6 banks).
  - exp -> e_sb ring (fp16, 2-chunk parity) feeds PV as the *moving*
    operand: acc^T [97, 512] += V~_j.T @ e_slice [128, 512], V~ tiles
    stationary. 512-row moving MMs fully hide the ~100ns LDWEIGHTS (a
    [128,128]-stationary PV variant measured 104ns/LDW exposed = +13us
    per chunk). V~'s ones column (from the projection) makes acc row 96
    the softmax denominator.
  - Epilogue per chunk: acc -> SBUF copy, 4 PE transposes [97,128] ->
    [128,97], DVE reciprocal + per-row mul, DMA out. PSUM budget: 6
    (scores) + 1 (acc) + 1 (transpose/G-seg) = 8 banks exactly.
  - Software pipeline: PV of chunk c-1 + its epilogue interleave into
    chunk c's score stream (MMs 5/group over groups 0-6, copy at 7,
    transposes at 8/9/10/next-0); h-slab transposes (batched 4 per PSUM
    round-trip) and V~ projection ride chunk 0; G~^T segment c+1 rides
    chunk c. PE stays dense so HAM holds the 2.4GHz pstate.
"""

import functools
import math

import numpy as np

import concourse.mybir as mybir
import concourse.tile as tile
from concourse import bacc
from concourse.bass import ts
from concourse.bass_utils import run_bass_kernel_spmd

S = 4096
D = 96
P = 128              # j/i tile (partition) size
NI = 512             # i-chunk width (columns per scores matmul)
JG = 3               # j-tiles per exp group (3 PSUM banks -> N=1536)
N_CORES = 8
F32 = mybir.dt.float32
F16 = mybir.dt.float16
AF = mybir.ActivationFunctionType


def build_attention_kernel(tc, out_dram, h, Wq, bq, Wk, bk, Wv, bv, s=S):
    nc = tc.nc
    nj = s // P                    # 32 j-tiles
    nchunks = s // NI              # 8 i-chunks
    ntile = NI // P                # 4 i-tiles per chunk
    njg = (nj + JG - 1) // JG      # 11 exp groups per chunk (last ragged)
    scale = 1.0 / math.sqrt(D)

    def jtiles(g):
        return range(g * JG, min((g + 1) * JG, nj))

    from contextlib import ExitStack
    with ExitStack() as ctx:
        singles = ctx.enter_context(tc.tile_pool(name="singles", bufs=1))
        hp = ctx.enter_context(tc.tile_pool(name="hp", bufs=8))
        op = ctx.enter_context(tc.tile_pool(name="op", bufs=4))
        # PSUM: 2 x 3-bank score slots + 1-bank acc + 1-bank misc = 8.
        scp = ctx.enter_context(tc.tile_pool(name="scp", bufs=2, space="PSUM"))
        accp = ctx.enter_context(
            tc.tile_pool(name="accp", bufs=1, space="PSUM"))
        trp = ctx.enter_context(tc.tile_pool(name="trp", bufs=1, space="PSUM"))

        ident_dram = nc.inline_tensor(np.eye(P, dtype=np.float32),
                                      name="ident_const")
        ident = singles.tile([P, P], F32)
        nc.sync.dma_start(out=ident, in_=ident_dram.ap())
        ones_dram = nc.inline_tensor(np.ones((1, s), dtype=np.float16),
                                     name="ones_row")

        # --- persistent SBUF tensors ---
        hT = singles.tile([D + 1, s], F16)        # h~^T (row 96 = ones)
        GT = singles.tile([D + 1, s], F16)        # G~^T = A~^T h~^T
        Vt = singles.tile([P, nj, D + 1], F16)    # V~ tiles (col 96 = ones)
        # e_sb ring: [chunk parity][group][JG*NI cols] of exp(scores^T) fp16
        e_sb = singles.tile([P, 2, njg, JG * NI], F16)
        ones_col = singles.tile([P, 1], F32)
        nc.vector.memset(ones_col, 1.0)
        nc.sync.dma_start(out=hT[D:D + 1, :], in_=ones_dram.ap())

        # --- weights: augmented transposed forms, no host-side prep ---
        # waq/wak [96, 97] f32: cols 0-95 = Wq/Wk (natural), col 96 = bias.
        # R [97, 97] fp16: rows 0-95 = Wv^T, row 96 = bv, col 96 = e_96.
        waq = singles.tile([D, D + 1], F32)
        wak = singles.tile([D, D + 1], F32)
        wv_sb = singles.tile([D, D], F32)
        bias_sb = singles.tile([2, D], F32)
        bv_sb = singles.tile([1, D], F32)
        nc.sync.dma_start(out=waq[:, 0:D], in_=Wq)
        nc.sync.dma_start(out=wak[:, 0:D], in_=Wk)
        nc.sync.dma_start(out=wv_sb, in_=Wv)
        nc.sync.dma_start(out=bias_sb[0:1, :], in_=bq.unsqueeze(0))
        nc.sync.dma_start(out=bias_sb[1:2, :], in_=bk.unsqueeze(0))
        nc.sync.dma_start(out=bv_sb, in_=bv.unsqueeze(0))

        # bias columns via one PE transpose of [2, 96]
        ps_b = trp.tile([D, 2], F32, tag="u")
        nc.tensor.transpose(ps_b, bias_sb, ident[0:2, 0:2])
        nc.vector.tensor_copy(waq[:, D:D + 1], ps_b[:, 0:1])
        nc.vector.tensor_copy(wak[:, D:D + 1], ps_b[:, 1:2])

        # A~ = (W~q W~k^T) * scale -> fp16 [97, 97]
        ps_a = accp.tile([D + 1, D + 1], F32, tag="u")
        nc.tensor.matmul(ps_a, lhsT=waq, rhs=wak, start=True, stop=True)
        A16 = singles.tile([D + 1, D + 1], F16)
        nc.vector.tensor_scalar_mul(A16, ps_a, scale)

        # R for the V~ projection
        R = singles.tile([D + 1, D + 1], F16)
        nc.vector.memset(R, 0.0)
        ps_w = trp.tile([D, D], F32, tag="u")
        nc.tensor.transpose(ps_w, wv_sb, ident[0:D, 0:D])
        nc.vector.tensor_copy(R[0:D, 0:D], ps_w)
        nc.vector.tensor_copy(R[D:D + 1, 0:D], bv_sb)
        nc.vector.tensor_copy(R[D:D + 1, D:D + 1], ones_col[0:1, 0:1])

        # --- batched prologue helpers (4 tiles per PSUM round-trip) ---
        pools = [accp, trp]

        def emit_tr4(q):
            # transpose h j-tiles 4q..4q+3 into hT via one wide PSUM tile
            pt = pools[q % 2].tile([D, 4 * P], F32, tag="u")
            for k in range(4):
                h_sb = hp.tile([P, D], F32, tag="h_sb")
                nc.sync.dma_start(out=h_sb, in_=h[ts(4 * q + k, P), :])
                nc.tensor.transpose(pt[:, ts(k, P)], h_sb, ident)
            nc.vector.tensor_copy(hT[0:D, ts(q, 4 * P)], pt)

        def emit_v4(q):
            # V~ projection for j-tiles 4q..4q+3
            pt = pools[(q + 1) % 2].tile([P, 4 * (D + 1)], F32, tag="u")
            for k in range(4):
                nc.tensor.matmul(pt[:, ts(k, D + 1)],
                                 lhsT=hT[:, ts(4 * q + k, P)], rhs=R,
                                 start=True, stop=True)
            nc.vector.tensor_copy(Vt[:, 4 * q:4 * q + 4, :], pt)

        def emit_g_seg(seg):
            ps_g = trp.tile([D + 1, NI], F32, tag="u")
            nc.tensor.matmul(ps_g, lhsT=A16, rhs=hT[:, ts(seg, NI)],
                             start=True, stop=True)
            nc.vector.tensor_copy(GT[:, ts(seg, NI)], ps_g)

        # PV + epilogue of chunk c as (group-offset-in-next-chunk, unit)
        def pv_units(c):
            par = c % 2
            units = []
            acc = accp.tile([D + 1, NI], F32, tag="u")
            for j in range(nj):
                g, jj = j // JG, j % JG
                units.append((j // 5, functools.partial(
                    nc.tensor.matmul, acc,
                    lhsT=Vt[:, j, :],
                    rhs=e_sb[:, par, g, ts(jj, NI)],
                    start=(j == 0), stop=(j == nj - 1))))
            eoT = op.tile([D + 1, NI], F32, tag="eoT")
            units.append((7, functools.partial(nc.vector.tensor_copy,
                                               eoT, acc)))

            def epi(k, c=c, eoT=eoT):
                ps_tr = trp.tile([P, D + 1], F32, tag="u")
                nc.tensor.transpose(ps_tr, eoT[:, ts(k, P)],
                                    ident[0:D + 1, 0:D + 1])
                rec = op.tile([P, 1], F32, tag="rec")
                nc.vector.reciprocal(rec, ps_tr[:, D:D + 1])
                o_sb = op.tile([P, D], F32, tag="o_sb")
                nc.vector.tensor_scalar_mul(o_sb, ps_tr[:, 0:D], rec)
                i0 = c * NI + k * P
                nc.sync.dma_start(out=out_dram[i0:i0 + P, :], in_=o_sb)
            for k in range(ntile):
                units.append((8 + k, functools.partial(epi, k)))
            return units

        # --- prologue: enough for chunk 0 group 0 ---
        emit_tr4(0)
        emit_g_seg(0)

        # --- main loop ---
        pending = []          # (abs_group, unit) for PV interleave
        abs_g = 0
        for c in range(nchunks):
            if c > 0:
                pending.extend((abs_g + off, u) for off, u in pv_units(c - 1))
            for g in range(njg):
                jts = list(jtiles(g))
                sc = scp.tile([P, JG * NI], F32, tag="sc")
                for jj, jt in enumerate(jts):
                    nc.tensor.matmul(sc[:, ts(jj, NI)],
                                     lhsT=hT[:, ts(jt, P)],
                                     rhs=GT[:, ts(c, NI)],
                                     start=True, stop=True)
                width = len(jts) * NI
                nc.scalar.activation(out=e_sb[:, c % 2, g, 0:width],
                                     in_=sc[:, 0:width], func=AF.Exp)
                # interleaved extras keep PE dense without starving ACT
                if c == 0:
                    if g < 7:
                        emit_tr4(g + 1)
                    if g < 8:
                        emit_v4(g)
                if g == 5 and c + 1 < nchunks:
                    emit_g_seg(c + 1)
                left = []
                for ag, u in pending:
                    (u() if ag <= abs_g else left.append((ag, u)))
                pending = left
                abs_g += 1
        # tail: drain pending epilogue, then PV of the last chunk
        for _, u in sorted(pending, key=lambda x: x[0]):
            u()
        for _, u in pv_units(nchunks - 1):
            u()


@functools.lru_cache(maxsize=None)
def _build_module(s=S):
    nc = bacc.Bacc("TRN2", target_bir_lowering=False, debug=False,
                   num_devices=N_CORES)
    h = nc.dram_tensor("h", [s, D], F32, kind="ExternalInput").ap()
    Wq = nc.dram_tensor("Wq", [D, D], F32, kind="ExternalInput").ap()
    bq = nc.dram_tensor("bq", [D], F32, kind="ExternalInput").ap()
    Wk = nc.dram_tensor("Wk", [D, D], F32, kind="ExternalInput").ap()
    bk = nc.dram_tensor("bk", [D], F32, kind="ExternalInput").ap()
    Wv = nc.dram_tensor("Wv", [D, D], F32, kind="ExternalInput").ap()
    bv = nc.dram_tensor("bv", [D], F32, kind="ExternalInput").ap()
    out = nc.dram_tensor("out", [s, D], F32, kind="ExternalOutput").ap()
    with tile.TileContext(nc) as tc:
        build_attention_kernel(tc, out, h, Wq, bq, Wk, bk, Wv, bv, s=s)
    nc.compile()
    return nc


def _run(inputs, trace=False):
    nc = _build_module(S)
    arrs = {k: np.ascontiguousarray(np.asarray(v), dtype=np.float32)
            for k, v in inputs.items()}
    in_maps = []
    for b_ in range(N_CORES):
        in_maps.append({
            "h": arrs["h"][b_],
            "Wq": arrs["Wq"], "bq": arrs["bq"],
            "Wk": arrs["Wk"], "bk": arrs["bk"],
            "Wv": arrs["Wv"], "bv": arrs["bv"],
        })
    res = run_bass_kernel_spmd(nc, in_maps, core_ids=list(range(N_CORES)),
                               trace=trace)
    out = np.stack([res.results[b_]["out"] for b_ in range(N_CORES)], axis=0)
    return out, res


def kernel(**inputs):
    out, _ = _run(inputs, trace=False)
    return out


def kernel_profiled(trace=True, **inputs):
    out, res = _run(inputs, trace=trace)
    return out, res
